# revision 9
# baseline (speedup 1.0000x reference)
"""Trainium2 Bass kernel for ChannelPatchEmbedding (dense_cnn).

Computes, for x:(B,C,64,64):
    out[b, c*256 + f*16 + t0, e] =
        sum_{u,v} x[b,c,4f+u,4t0+v] * W[e,u,v] + bias[e]
        + channel_embed[c,e] + spatial_embed[spatial_idx[c],e]
        + freq_pos[f,e] + time_pos[t0,e]

Sharding: pure data parallel over the batch dim across 8 NeuronCores.

End-to-end wall time is dominated by the axon tunnel, whose ~50MB/s
capacity is shared between directions, so the implementation minimizes
total bytes moved and per-call overhead:

  - x ships as int8 (xq = round(x/sx), 1.13MB/core); the device widens
    it to bf16. The conv weights ship as a 6KB bf16 table holding
    W^T * sx / s6 and are placed on the s-block-diagonal on device, so
    one matmul computes 8 patch-octets and PSUM holds conv/s6 with
    |psum| <= 31.45 guaranteed by a Cauchy-Schwarz bound on the host
    (max patch 2-norm x max filter 2-norm): the 6-bit quantization
    below never clips. The lg upload is issued as an async device_put
    before the scale computation so the h2d overlaps host work.
  - Per 1024-patch group: 3 matmuls (N=512, one PSUM bank each), then
    DVE evicts U = uint8(psum + 32) (6-bit codes 1..63) and packs U
    into two separate streams: 4-bit nibbles (96B/patch) and 2-bit
    fields (48B/patch): 0.75 bytes/value, 84.9MB total, max quant
    error s6/2 -> rel err ~3e-3, 6x under tolerance.
  - Stores are contiguous DMAs alternating the two HWDGE rings.
  - The jitted PJRT executable is built once and cached; donated output
    buffers are created on-device (no zero upload); shards are fetched
    with copy_to_host_async and decoded into the final f32 buffer by a
    thread pool while later shards are still in flight. The decode uses
    only ufunc arithmetic (shifts/masks), which releases the GIL, so it
    genuinely parallelizes and hides under the transfer.
"""

from concurrent.futures import ThreadPoolExecutor

import numpy as np
import ml_dtypes
import jax
import jax.numpy as jnp
from jax.experimental.shard_map import shard_map
from jax.sharding import Mesh, NamedSharding, PartitionSpec

import concourse.bass as bass
import concourse.mybir as mybir
from concourse import bass2jax
from concourse.tile import TileContext
from concourse.vector_clock import ScopedClock

f32 = mybir.dt.float32
bf16 = mybir.dt.bfloat16
i8 = mybir.dt.int8
u8 = mybir.dt.uint8
ALU = mybir.AluOpType

B, C, FR, T = 256, 9, 64, 64
P, E = 4, 192
NF = NT = 16
N_PATCH = C * NF * NT  # 2304
N_CORES = 8
BPC = B // N_CORES  # 32
NQ = BPC // 4  # 8 batch-quads per core
NGROUP = 72  # per core: 8 quads x (4 batches x {A,B} + C)
W8 = 8 * E  # 1536 psum cols per group
HB = 96  # hi-nibble bytes per patch
LB = 48  # lo-2bit bytes per patch


class _TC(TileContext):
    """TileContext whose kernel-tail drain never carries more than one
    sync-wait: the walrus build in this container rejects multi-wait CTRL
    instructions, and the stock tail Drain aggregates every residual
    proc wait onto itself. Spread them across single-wait SP nops."""

    def _drain_and_barrier(self, tick_clock, wait_clock):
        probe = self.nc.sync.nop()
        wait_clock.add_sem_waits(
            probe.ins, ScopedClock({None: tick_clock.global_clock})
        )
        si = probe.ins.sync_info
        waits = list(si.on_wait) if si is not None and si.on_wait else []
        if len(waits) > 1:
            si.on_wait = waits[:1]
            for w in waits[1:]:
                n2 = self.nc.sync.nop()
                si2 = n2.ins.sync_info
                if si2 is None:
                    n2.ins.sync_info = mybir.SyncInfo(on_wait=[w], on_update=[])
                else:
                    si2.on_wait = [w]
        self.nc.sync.drain()
        self.nc.all_engine_barrier()
        popped = self.nc._tile_sem_poison_stack.pop()
        assert popped is self._sem_poison
        self.nc.clear_and_free_semaphores(list(self.sems.allocated().values()))
        self.nc.all_engine_barrier()


def _split_multi_waits(nc: bass.Bass, max_waits: int = 1) -> None:
    """This container's walrus rejects instructions carrying more than one
    sync-wait. Move excess waits onto same-engine NoOps inserted right
    before the instruction (equivalent semantics: the sequencer blocks on
    each in turn)."""
    for fn in nc.m.functions:
        for blk in fn.blocks:
            out, changed = [], False
            for inst in list(blk.instructions):
                si = inst.sync_info
                if si is not None and si.on_wait and len(si.on_wait) > max_waits:
                    waits = list(si.on_wait)
                    for i, w in enumerate(waits[:-max_waits]):
                        out.append(
                            mybir.InstNoOp(
                                name=f"{inst.name}-wsplit{i}",
                                engine=inst.engine,
                                sync_info=mybir.SyncInfo(
                                    on_wait=[w], on_update=[]
                                ),
                            )
                        )
                    si.on_wait = waits[-max_waits:]
                    changed = True
                out.append(inst)
            if changed:
                blk.instructions = out


def build_nc() -> bass.Bass:
    nc = bass.Bass(trn_type="TRN2", debug=False)

    lgq = nc.dram_tensor("lgq", [128, NGROUP * 128], i8, kind="ExternalInput")
    wt_d = nc.dram_tensor("wt", [16, E], bf16, kind="ExternalInput")
    outh = nc.dram_tensor("outh", [BPC, N_PATCH, HB], u8, kind="ExternalOutput")
    outl = nc.dram_tensor("outl", [BPC, N_PATCH, LB], u8, kind="ExternalOutput")

    outha = outh.ap()
    outla = outl.ap()

    with _TC(nc) as tc:
        with (
            tc.tile_pool(name="const", bufs=1) as cp,
            tc.tile_pool(name="outph", bufs=8) as outph,
            tc.tile_pool(name="outpl", bufs=8) as outpl,
            tc.tile_pool(name="outch", bufs=3) as outch,
            tc.tile_pool(name="outcl", bufs=3) as outcl,
            tc.tile_pool(name="upool", bufs=3) as upool,
            tc.tile_pool(name="vpool", bufs=4) as vpool,
            tc.tile_pool(name="mpool", bufs=2) as mpool,
            tc.tile_pool(name="psum", bufs=2, space="PSUM") as psp,
        ):
            # block-diagonal weights, built on device from the 6KB table
            rhs_sb = cp.tile([128, W8], bf16, name="rhs_sb")
            wt_sb = cp.tile([16, E], bf16, name="wt_sb")
            nc.vector.memset(rhs_sb[:, :], 0)
            nc.gpsimd.dma_start(out=wt_sb[:, :], in_=wt_d.ap())
            for s in range(8):
                nc.gpsimd.dma_start(
                    out=rhs_sb[16 * s : 16 * s + 16, E * s : E * (s + 1)],
                    in_=wt_sb[:, :],
                )
            # lgq in sixths, alternating rings; DVE widens int8 -> bf16
            lgts = []
            LW = NGROUP * 128 // 6
            for li in range(6):
                tq = cp.tile([128, LW], i8, name=f"lgq{li}")
                t = cp.tile([128, LW], bf16, name=f"lgt{li}")
                (nc.sync if li % 2 == 0 else nc.scalar).dma_start(
                    out=tq[:, :], in_=lgq.ap()[:, li * LW : (li + 1) * LW]
                )
                nc.vector.tensor_copy(out=t[:, :], in_=tq[:, :])
                lgts.append(t)

            def mm_group(g: int, oth, otl, hcol0: int, lcol0: int):
                """3 matmuls -> psum = conv/s6; evict U = uint8(psum+32)
                (values 1..63); pack nibbles into oth, 2-bit fields into
                otl."""
                gl = g % 12
                lhs = lgts[g // 12][:, 128 * gl : 128 * (gl + 1)]
                ps = psp.tile([128, W8], f32, name="ps")
                for p3 in range(3):
                    nc.tensor.matmul(
                        out=ps[:, 512 * p3 : 512 * (p3 + 1)],
                        lhsT=lhs,
                        rhs=rhs_sb[:, 512 * p3 : 512 * (p3 + 1)],
                        start=True,
                        stop=True,
                    )
                U = upool.tile([128, W8], u8, name="U")
                nc.vector.tensor_scalar(
                    out=U[:, :], in0=ps[:, :], scalar1=32.0, scalar2=None,
                    op0=ALU.add,
                )
                V = vpool.tile([128, W8], u8, name="V")  # U >> 2 (4-bit)
                nc.vector.tensor_scalar(
                    out=V[:, :], in0=U[:, :], scalar1=2, scalar2=None,
                    op0=ALU.logical_shift_right,
                )
                L = vpool.tile([128, W8], u8, name="L")  # U & 3 (2-bit)
                nc.vector.tensor_scalar(
                    out=L[:, :], in0=U[:, :], scalar1=3, scalar2=None,
                    op0=ALU.bitwise_and,
                )
                m1 = mpool.tile([128, 2 * 384], u8, name="m1")
                for s in range(8):
                    e0 = E * s
                    # hi: nibble pairs -> bytes (even e in high nibble)
                    nc.vector.scalar_tensor_tensor(
                        out=oth[:, hcol0 + HB * s : hcol0 + HB * (s + 1)],
                        in0=V[:, e0 : e0 + E : 2], scalar=16.0,
                        in1=V[:, e0 + 1 : e0 + E : 2],
                        op0=ALU.mult, op1=ALU.add,
                    )
                    # lo: 2-bit quads -> bytes (e%4==0 in bits 7:6, ==3 in 1:0)
                    nc.vector.scalar_tensor_tensor(
                        out=m1[:, 48 * s : 48 * s + 48],
                        in0=L[:, e0 : e0 + E : 4], scalar=4.0,
                        in1=L[:, e0 + 1 : e0 + E : 4],
                        op0=ALU.mult, op1=ALU.add,
                    )
                    nc.vector.scalar_tensor_tensor(
                        out=m1[:, 384 + 48 * s : 384 + 48 * s + 48],
                        in0=L[:, e0 + 2 : e0 + E : 4], scalar=4.0,
                        in1=L[:, e0 + 3 : e0 + E : 4],
                        op0=ALU.mult, op1=ALU.add,
                    )
                for s in range(8):
                    nc.vector.scalar_tensor_tensor(
                        out=otl[:, lcol0 + LB * s : lcol0 + LB * (s + 1)],
                        in0=m1[:, 48 * s : 48 * s + 48], scalar=16.0,
                        in1=m1[:, 384 + 48 * s : 384 + 48 * s + 48],
                        op0=ALU.mult, op1=ALU.add,
                    )

            g = 0
            store_engs = (nc.sync, nc.scalar)
            dma_flip = 0
            for bq in range(NQ):
                for bl in range(4):
                    b = 4 * bq + bl
                    oth = outph.tile([128, 2 * 8 * HB], u8, name="oth")
                    otl = outpl.tile([128, 2 * 8 * LB], u8, name="otl")
                    mm_group(g, oth, otl, 0, 0)  # A: channels 0-3
                    g += 1
                    mm_group(g, oth, otl, 8 * HB, 8 * LB)  # B: channels 4-7
                    g += 1
                    dsth = outha[b, 0 : 2 * 1024, :].rearrange(
                        "(blk m s) j -> m blk (s j)", blk=2, s=8
                    )
                    store_engs[dma_flip % 2].dma_start(out=dsth, in_=oth[:, :])
                    dma_flip += 1
                    dstl = outla[b, 0 : 2 * 1024, :].rearrange(
                        "(blk m s) j -> m blk (s j)", blk=2, s=8
                    )
                    store_engs[dma_flip % 2].dma_start(out=dstl, in_=otl[:, :])
                    dma_flip += 1
                # C: channel 8 of the 4 batches in this quad
                oth = outch.tile([128, 8 * HB], u8, name="otch")
                otl = outcl.tile([128, 8 * LB], u8, name="otcl")
                mm_group(g, oth, otl, 0, 0)
                g += 1
                dsth = outha[4 * bq : 4 * bq + 4, 8 * 256 : 9 * 256, :].rearrange(
                    "j (r s) e -> j r (s e)", s=8
                )
                store_engs[dma_flip % 2].dma_start(out=dsth, in_=oth[:, :])
                dma_flip += 1
                dstl = outla[4 * bq : 4 * bq + 4, 8 * 256 : 9 * 256, :].rearrange(
                    "j (r s) e -> j r (s e)", s=8
                )
                store_engs[dma_flip % 2].dma_start(out=dstl, in_=otl[:, :])
                dma_flip += 1
            assert g == NGROUP

    _split_multi_waits(nc)
    return nc


def _marshal_lg(xq: np.ndarray) -> np.ndarray:
    """Build the global lhsT input (N_CORES*128, NGROUP*128) int8 from the
    int8-quantized x.

    Layout per core: lg[k, g*128 + m], k=(s,u,v), groups ordered
    [A(b0),B(b0),...,A(b3),B(b3),C] per batch-quad; m=(c%4|j, f, h).
    x[b, c, 4f+u, 32h+4s+v]."""
    lgt = np.empty((N_CORES, 128, NQ, 9, 128), np.int8)
    lv = lgt.reshape(N_CORES, 8, P, P, NQ, 9, 128)  # k -> (s, u, v)
    # A/B groups: channels 0-7
    # axes: (core, bq, bl, ab, c4, f, u, h, s, v)
    xab = xq[:, 0:8].reshape(N_CORES, NQ, 4, 2, 4, NF, P, 2, 8, P)
    lv[:, :, :, :, :, 0:8, :] = (
        xab.transpose(0, 8, 6, 9, 1, 2, 3, 4, 5, 7)  # core,s,u,v,bq,bl,ab,c4,f,h
        .reshape(N_CORES, 8, P, P, NQ, 8, 128)
    )
    # C groups: channel 8, m=(j=batch lane, f, h)
    # axes: (core, bq, j, f, u, h, s, v)
    xc = xq[:, 8].reshape(N_CORES, NQ, 4, NF, P, 2, 8, P)
    lv[:, :, :, :, :, 8, :] = (
        xc.transpose(0, 6, 4, 7, 1, 2, 3, 5)  # core,s,u,v,bq,j,f,h
        .reshape(N_CORES, 8, P, P, NQ, 128)
    )
    return lgt.reshape(N_CORES * 128, NGROUP * 128)


def _embed_table(b, channel_embed, spatial_embed, time_pos, freq_pos,
                 spatial_idx) -> np.ndarray:
    """emb[p=(c,f,t), e]: everything except the conv, f32 (2304, 192)."""
    chs = channel_embed + spatial_embed[spatial_idx] + b[None, :]  # (C, E)
    emb = (chs[:, None, None, :]
           + freq_pos[None, :, None, :]
           + time_pos[None, None, :, :])
    return np.ascontiguousarray(emb.reshape(N_PATCH, E), dtype=np.float32)


class _Exec:
    """One-time build: Bass module -> cached jitted PJRT executable."""

    def __init__(self):
        bass2jax.install_neuronx_cc_hook()
        nc = build_nc()
        self.nc = nc
        partition_name = (
            nc.partition_id_tensor.name if nc.partition_id_tensor else None
        )
        in_names: list[str] = []
        out_names: list[str] = []
        out_avals: list[jax.core.ShapedArray] = []
        for alloc in nc.m.functions[0].allocations:
            if not isinstance(alloc, mybir.MemoryLocationSet):
                continue
            name = alloc.memorylocations[0].name
            if alloc.kind == "ExternalInput":
                if name != partition_name:
                    in_names.append(name)
            elif alloc.kind == "ExternalOutput":
                out_names.append(name)
                out_avals.append(
                    jax.core.ShapedArray(
                        tuple(alloc.tensor_shape), mybir.dt.np(alloc.dtype)
                    )
                )
        n_params = len(in_names)
        n_outs = len(out_avals)
        self.in_names = list(in_names)
        self.out_names = list(out_names)
        in_names = in_names + out_names
        if partition_name is not None:
            in_names.append(partition_name)

        def _body(*args):
            operands = list(args)
            if partition_name is not None:
                operands.append(bass2jax.partition_id_tensor())
            outs = bass2jax._bass_exec_p.bind(
                *operands,
                out_avals=tuple(out_avals),
                in_names=tuple(in_names),
                out_names=tuple(out_names),
                lowering_input_output_aliases=(),
                sim_require_finite=True,
                sim_require_nnan=True,
                nc=nc,
            )
            return tuple(outs)

        devices = jax.devices()[:N_CORES]
        assert len(devices) == N_CORES, len(jax.devices())
        self.mesh = Mesh(np.asarray(devices), ("core",))
        spec = PartitionSpec("core")
        self.sharding = NamedSharding(self.mesh, spec)
        self.sharded = jax.jit(
            shard_map(
                _body,
                mesh=self.mesh,
                in_specs=(spec,) * (n_params + n_outs),
                out_specs=(spec,) * n_outs,
                check_rep=False,
            ),
            donate_argnums=tuple(range(n_params, n_params + n_outs)),
            keep_unused=True,
        )
        # donated output buffers, created on-device (no 85MB zero upload)
        self.zeros_fn = jax.jit(
            shard_map(
                lambda: (
                    jnp.zeros((BPC, N_PATCH, HB), jnp.uint8),
                    jnp.zeros((BPC, N_PATCH, LB), jnp.uint8),
                ),
                mesh=self.mesh,
                in_specs=(),
                out_specs=(spec, spec),
                check_rep=False,
            )
        )


_CACHE: dict = {}


def _get_exec() -> _Exec:
    if "exec" not in _CACHE:
        _CACHE["exec"] = _Exec()
        _CACHE["pool"] = ThreadPoolExecutor(N_CORES + 4)
    return _CACHE["exec"]


def kernel(**inputs: np.ndarray) -> np.ndarray:
    arrs = {k: np.asarray(v) for k, v in inputs.items()}
    x = arrs["x"].astype(np.float32, copy=False)
    assert x.shape == (B, C, FR, T), x.shape
    W = arrs["W"].astype(np.float32, copy=False)
    ex = _get_exec()

    # input quantization: xq = round(x / sx); upload starts immediately
    # (async) so the h2d overlaps the scale computation below
    xmax = float(max(x.max(), -x.min())) or 1.0
    sx = np.float32(xmax / 127.0)
    xq = np.rint(x * (1.0 / sx)).astype(np.int8)
    lg_dev = jax.device_put(_marshal_lg(xq), ex.sharding)

    # 6-bit output scale via Cauchy-Schwarz: |conv| <= ||x_p|| * ||W_e||;
    # patch norms computed exactly, quantization slack of 0.5 per tap.
    pn2 = np.square(x).reshape(B, C, NF, P, NT, P).sum(axis=(3, 5))
    qn = float(np.sqrt(pn2.max())) / float(sx) + 2.0  # max ||xq_p||_2 bound
    wtb = W.transpose(1, 2, 0).reshape(P * P, E)  # [(u,v), e]
    wn_f = float(np.sqrt(np.square(wtb).sum(axis=0).max()))
    s6 = np.float32(qn * float(sx) * wn_f / 30.9) if wn_f > 0 else np.float32(1.0)
    for _ in range(4):  # exact no-clip check on the bf16 weights actually sent
        wt16 = (wtb * (float(sx) / s6)).astype(ml_dtypes.bfloat16)
        wn = float(np.sqrt(np.square(wt16.astype(np.float32)).sum(axis=0).max()))
        if qn * wn <= 31.45:
            break
        s6 = np.float32(s6 * (qn * wn / 31.4))
    else:
        raise AssertionError("6-bit scale did not converge")
    wt_g = np.ascontiguousarray(
        np.broadcast_to(wt16, (N_CORES, 16, E)).reshape(N_CORES * 16, E)
    )

    emb = _embed_table(
        arrs["b"].astype(np.float32, copy=False),
        arrs["channel_embed"].astype(np.float32, copy=False),
        arrs["spatial_embed"].astype(np.float32, copy=False),
        arrs["time_pos"].astype(np.float32, copy=False),
        arrs["freq_pos"].astype(np.float32, copy=False),
        arrs["spatial_idx"],
    )
    s6f = np.float32(s6)
    emb_off = emb - np.float32(32.0) * s6f  # folds the +32 code offset

    zh, zl = ex.zeros_fn()
    hi_g, lo_g = ex.sharded(lg_dev, wt_g, zh, zl)

    # fetch shards in flight; decode with GIL-releasing ufunc arithmetic
    hi_shards = {s.index[0].start or 0: s for s in hi_g.addressable_shards}
    lo_shards = {s.index[0].start or 0: s for s in lo_g.addressable_shards}
    for s in hi_shards.values():
        s.data.copy_to_host_async()
    for s in lo_shards.values():
        s.data.copy_to_host_async()

    final = np.empty((B, N_PATCH, E), np.float32)
    CH = 8  # batches per decode task
    pool = _CACHE["pool"]

    def decode(ah, al, i0):
        hi = ah  # (CH, N_PATCH, HB) uint8, contiguous
        lo = al  # (CH, N_PATCH, LB)
        he = hi[..., 0::2]  # covers e%4 in {0,1}
        ho = hi[..., 1::2]  # covers e%4 in {2,3}
        Ub = np.empty((ah.shape[0], N_PATCH, E), np.uint8)
        t = np.right_shift(he, 4)
        np.left_shift(t, 2, out=t)
        t2 = np.right_shift(lo, 6)
        np.add(t, t2, out=t)
        Ub[..., 0::4] = t
        np.bitwise_and(he, 15, out=t)
        np.left_shift(t, 2, out=t)
        np.right_shift(lo, 4, out=t2)
        np.bitwise_and(t2, 3, out=t2)
        np.add(t, t2, out=t)
        Ub[..., 1::4] = t
        np.right_shift(ho, 4, out=t)
        np.left_shift(t, 2, out=t)
        np.right_shift(lo, 2, out=t2)
        np.bitwise_and(t2, 3, out=t2)
        np.add(t, t2, out=t)
        Ub[..., 2::4] = t
        np.bitwise_and(ho, 15, out=t)
        np.left_shift(t, 2, out=t)
        np.bitwise_and(lo, 3, out=t2)
        np.add(t, t2, out=t)
        Ub[..., 3::4] = t
        out_v = final[i0 : i0 + ah.shape[0]]
        np.multiply(Ub, s6f, out=out_v, casting="unsafe")
        np.add(out_v, emb_off[None], out=out_v)

    decode_futs = []

    def fetch(i0):
        ah = np.asarray(hi_shards[i0].data)
        al = np.asarray(lo_shards[i0].data)
        for j0 in range(0, BPC, CH):
            decode_futs.append(
                pool.submit(decode, ah[j0 : j0 + CH], al[j0 : j0 + CH], i0 + j0)
            )

    for f in [pool.submit(fetch, i0) for i0 in sorted(hi_shards)]:
        f.result()
    for f in decode_futs:
        f.result()
    return final


# revision 14
# speedup vs baseline: 1.3988x; 1.3988x over previous
"""Trainium2 Bass kernel for ChannelPatchEmbedding (dense_cnn).

Computes, for x:(B,C,64,64):
    out[b, c*256 + f*16 + t0, e] =
        sum_{u,v} x[b,c,4f+u,4t0+v] * W[e,u,v] + bias[e]
        + channel_embed[c,e] + spatial_embed[spatial_idx[c],e]
        + freq_pos[f,e] + time_pos[t0,e]

Sharding: pure data parallel over the batch dim across 8 NeuronCores.

End-to-end wall time is dominated by the axon tunnel, whose ~50MB/s
capacity is shared between directions, so the implementation minimizes
total bytes moved and per-call overhead:

  - x ships as int8 (xq = round(x/sx), 1.13MB/core); the device widens
    it to bf16. The conv weights ship as a 6KB bf16 table holding
    W^T * sx / s6 and are placed on the s-block-diagonal on device, so
    one matmul computes 8 patch-octets and PSUM holds conv/s6 with
    |psum| <= 31.45 guaranteed by a Cauchy-Schwarz bound on the host
    (max patch 2-norm x max filter 2-norm): the 6-bit quantization
    below never clips. The lg upload is issued as an async device_put
    before the scale computation so the h2d overlaps host work.
  - Per 1024-patch group: 3 matmuls (N=512, one PSUM bank each), then
    DVE evicts U = uint8(psum + 32) (6-bit codes 1..63) and packs U
    into two separate streams: 4-bit nibbles (96B/patch) and 2-bit
    fields (48B/patch): 0.75 bytes/value, 84.9MB total, max quant
    error s6/2 -> rel err ~3e-3, 6x under tolerance.
  - Stores are contiguous DMAs alternating the two HWDGE rings.
  - The jitted PJRT executable is built once and cached; donated output
    buffers are created on-device (no zero upload); shards are fetched
    with copy_to_host_async and decoded into the final f32 buffer by a
    thread pool while later shards are still in flight. The decode uses
    only ufunc arithmetic (shifts/masks), which releases the GIL, so it
    genuinely parallelizes and hides under the transfer.
"""

from concurrent.futures import ThreadPoolExecutor

import numpy as np
import ml_dtypes
import jax
import jax.numpy as jnp
from jax.experimental.shard_map import shard_map
from jax.sharding import Mesh, NamedSharding, PartitionSpec

import concourse.bass as bass
import concourse.mybir as mybir
from concourse import bass2jax
from concourse.tile import TileContext
from concourse.vector_clock import ScopedClock

f32 = mybir.dt.float32
bf16 = mybir.dt.bfloat16
i8 = mybir.dt.int8
u8 = mybir.dt.uint8
ALU = mybir.AluOpType

B, C, FR, T = 256, 9, 64, 64
P, E = 4, 192
NF = NT = 16
N_PATCH = C * NF * NT  # 2304
N_CORES = 8
BPC = B // N_CORES  # 32
NQ = BPC // 4  # 8 batch-quads per core
NGROUP = 72  # per core: 8 quads x (4 batches x {A,B} + C)
W8 = 8 * E  # 1536 psum cols per group
HB = 96  # hi-nibble bytes per patch
LB = 48  # lo-2bit bytes per patch


class _TC(TileContext):
    """TileContext whose kernel-tail drain never carries more than one
    sync-wait: the walrus build in this container rejects multi-wait CTRL
    instructions, and the stock tail Drain aggregates every residual
    proc wait onto itself. Spread them across single-wait SP nops."""

    def _drain_and_barrier(self, tick_clock, wait_clock):
        probe = self.nc.sync.nop()
        wait_clock.add_sem_waits(
            probe.ins, ScopedClock({None: tick_clock.global_clock})
        )
        si = probe.ins.sync_info
        waits = list(si.on_wait) if si is not None and si.on_wait else []
        if len(waits) > 1:
            si.on_wait = waits[:1]
            for w in waits[1:]:
                n2 = self.nc.sync.nop()
                si2 = n2.ins.sync_info
                if si2 is None:
                    n2.ins.sync_info = mybir.SyncInfo(on_wait=[w], on_update=[])
                else:
                    si2.on_wait = [w]
        self.nc.sync.drain()
        self.nc.all_engine_barrier()
        popped = self.nc._tile_sem_poison_stack.pop()
        assert popped is self._sem_poison
        self.nc.clear_and_free_semaphores(list(self.sems.allocated().values()))
        self.nc.all_engine_barrier()


def _split_multi_waits(nc: bass.Bass, max_waits: int = 1) -> None:
    """This container's walrus rejects instructions carrying more than one
    sync-wait. Move excess waits onto same-engine NoOps inserted right
    before the instruction (equivalent semantics: the sequencer blocks on
    each in turn)."""
    for fn in nc.m.functions:
        for blk in fn.blocks:
            out, changed = [], False
            for inst in list(blk.instructions):
                si = inst.sync_info
                if si is not None and si.on_wait and len(si.on_wait) > max_waits:
                    waits = list(si.on_wait)
                    for i, w in enumerate(waits[:-max_waits]):
                        out.append(
                            mybir.InstNoOp(
                                name=f"{inst.name}-wsplit{i}",
                                engine=inst.engine,
                                sync_info=mybir.SyncInfo(
                                    on_wait=[w], on_update=[]
                                ),
                            )
                        )
                    si.on_wait = waits[-max_waits:]
                    changed = True
                out.append(inst)
            if changed:
                blk.instructions = out


def build_nc() -> bass.Bass:
    nc = bass.Bass(trn_type="TRN2", debug=False)

    lgq = nc.dram_tensor("lgq", [128, NGROUP * 128], i8, kind="ExternalInput")
    wt_d = nc.dram_tensor("wt", [16, E], bf16, kind="ExternalInput")
    outh = nc.dram_tensor("outh", [BPC, N_PATCH, HB], u8, kind="ExternalOutput")
    outl = nc.dram_tensor("outl", [BPC, N_PATCH, LB], u8, kind="ExternalOutput")

    outha = outh.ap()
    outla = outl.ap()

    with _TC(nc) as tc:
        with (
            tc.tile_pool(name="const", bufs=1) as cp,
            tc.tile_pool(name="outph", bufs=8) as outph,
            tc.tile_pool(name="outpl", bufs=8) as outpl,
            tc.tile_pool(name="outch", bufs=3) as outch,
            tc.tile_pool(name="outcl", bufs=3) as outcl,
            tc.tile_pool(name="upool", bufs=3) as upool,
            tc.tile_pool(name="vpool", bufs=4) as vpool,
            tc.tile_pool(name="mpool", bufs=2) as mpool,
            tc.tile_pool(name="psum", bufs=2, space="PSUM") as psp,
        ):
            # block-diagonal weights, built on device from the 6KB table
            rhs_sb = cp.tile([128, W8], bf16, name="rhs_sb")
            wt_sb = cp.tile([16, E], bf16, name="wt_sb")
            nc.vector.memset(rhs_sb[:, :], 0)
            nc.gpsimd.dma_start(out=wt_sb[:, :], in_=wt_d.ap())
            for s in range(8):
                nc.gpsimd.dma_start(
                    out=rhs_sb[16 * s : 16 * s + 16, E * s : E * (s + 1)],
                    in_=wt_sb[:, :],
                )
            # lgq in sixths, alternating rings; DVE widens int8 -> bf16
            lgts = []
            LW = NGROUP * 128 // 6
            for li in range(6):
                tq = cp.tile([128, LW], i8, name=f"lgq{li}")
                t = cp.tile([128, LW], bf16, name=f"lgt{li}")
                (nc.sync if li % 2 == 0 else nc.scalar).dma_start(
                    out=tq[:, :], in_=lgq.ap()[:, li * LW : (li + 1) * LW]
                )
                nc.vector.tensor_copy(out=t[:, :], in_=tq[:, :])
                lgts.append(t)

            def mm_group(g: int, oth, otl, hcol0: int, lcol0: int):
                """3 matmuls -> psum = conv/s6; evict U = uint8(psum+32)
                (values 1..63); pack nibbles into oth, 2-bit fields into
                otl."""
                gl = g % 12
                lhs = lgts[g // 12][:, 128 * gl : 128 * (gl + 1)]
                ps = psp.tile([128, W8], f32, name="ps")
                for p3 in range(3):
                    nc.tensor.matmul(
                        out=ps[:, 512 * p3 : 512 * (p3 + 1)],
                        lhsT=lhs,
                        rhs=rhs_sb[:, 512 * p3 : 512 * (p3 + 1)],
                        start=True,
                        stop=True,
                    )
                U = upool.tile([128, W8], u8, name="U")
                nc.vector.tensor_scalar(
                    out=U[:, :], in0=ps[:, :], scalar1=32.0, scalar2=None,
                    op0=ALU.add,
                )
                V = vpool.tile([128, W8], u8, name="V")  # U >> 2 (4-bit)
                nc.vector.tensor_scalar(
                    out=V[:, :], in0=U[:, :], scalar1=2, scalar2=None,
                    op0=ALU.logical_shift_right,
                )
                L = vpool.tile([128, W8], u8, name="L")  # U & 3 (2-bit)
                nc.vector.tensor_scalar(
                    out=L[:, :], in0=U[:, :], scalar1=3, scalar2=None,
                    op0=ALU.bitwise_and,
                )
                m1 = mpool.tile([128, 2 * 384], u8, name="m1")
                for s in range(8):
                    e0 = E * s
                    # hi: nibble pairs -> bytes (even e in high nibble)
                    nc.vector.scalar_tensor_tensor(
                        out=oth[:, hcol0 + HB * s : hcol0 + HB * (s + 1)],
                        in0=V[:, e0 : e0 + E : 2], scalar=16.0,
                        in1=V[:, e0 + 1 : e0 + E : 2],
                        op0=ALU.mult, op1=ALU.add,
                    )
                    # lo: 2-bit quads -> bytes (e%4==0 in bits 7:6, ==3 in 1:0)
                    nc.vector.scalar_tensor_tensor(
                        out=m1[:, 48 * s : 48 * s + 48],
                        in0=L[:, e0 : e0 + E : 4], scalar=4.0,
                        in1=L[:, e0 + 1 : e0 + E : 4],
                        op0=ALU.mult, op1=ALU.add,
                    )
                    nc.vector.scalar_tensor_tensor(
                        out=m1[:, 384 + 48 * s : 384 + 48 * s + 48],
                        in0=L[:, e0 + 2 : e0 + E : 4], scalar=4.0,
                        in1=L[:, e0 + 3 : e0 + E : 4],
                        op0=ALU.mult, op1=ALU.add,
                    )
                for s in range(8):
                    nc.vector.scalar_tensor_tensor(
                        out=otl[:, lcol0 + LB * s : lcol0 + LB * (s + 1)],
                        in0=m1[:, 48 * s : 48 * s + 48], scalar=16.0,
                        in1=m1[:, 384 + 48 * s : 384 + 48 * s + 48],
                        op0=ALU.mult, op1=ALU.add,
                    )

            g = 0
            store_engs = (nc.sync, nc.scalar)
            dma_flip = 0
            for bq in range(NQ):
                for bl in range(4):
                    b = 4 * bq + bl
                    oth = outph.tile([128, 2 * 8 * HB], u8, name="oth")
                    otl = outpl.tile([128, 2 * 8 * LB], u8, name="otl")
                    mm_group(g, oth, otl, 0, 0)  # A: channels 0-3
                    g += 1
                    mm_group(g, oth, otl, 8 * HB, 8 * LB)  # B: channels 4-7
                    g += 1
                    dsth = outha[b, 0 : 2 * 1024, :].rearrange(
                        "(blk m s) j -> m blk (s j)", blk=2, s=8
                    )
                    store_engs[dma_flip % 2].dma_start(out=dsth, in_=oth[:, :])
                    dma_flip += 1
                    dstl = outla[b, 0 : 2 * 1024, :].rearrange(
                        "(blk m s) j -> m blk (s j)", blk=2, s=8
                    )
                    store_engs[dma_flip % 2].dma_start(out=dstl, in_=otl[:, :])
                    dma_flip += 1
                # C: channel 8 of the 4 batches in this quad
                oth = outch.tile([128, 8 * HB], u8, name="otch")
                otl = outcl.tile([128, 8 * LB], u8, name="otcl")
                mm_group(g, oth, otl, 0, 0)
                g += 1
                dsth = outha[4 * bq : 4 * bq + 4, 8 * 256 : 9 * 256, :].rearrange(
                    "j (r s) e -> j r (s e)", s=8
                )
                store_engs[dma_flip % 2].dma_start(out=dsth, in_=oth[:, :])
                dma_flip += 1
                dstl = outla[4 * bq : 4 * bq + 4, 8 * 256 : 9 * 256, :].rearrange(
                    "j (r s) e -> j r (s e)", s=8
                )
                store_engs[dma_flip % 2].dma_start(out=dstl, in_=otl[:, :])
                dma_flip += 1
            assert g == NGROUP

    _split_multi_waits(nc)
    return nc


def _marshal_lg(xq: np.ndarray) -> np.ndarray:
    """Build the global lhsT input (N_CORES*128, NGROUP*128) int8 from the
    int8-quantized x.

    Layout per core: lg[k, g*128 + m], k=(s,u,v), groups ordered
    [A(b0),B(b0),...,A(b3),B(b3),C] per batch-quad; m=(c%4|j, f, h).
    x[b, c, 4f+u, 32h+4s+v]."""
    lgt = np.empty((N_CORES, 128, NQ, 9, 128), np.int8)
    lv = lgt.reshape(N_CORES, 8, P, P, NQ, 9, 128)  # k -> (s, u, v)
    # A/B groups: channels 0-7
    # axes: (core, bq, bl, ab, c4, f, u, h, s, v)
    xab = xq[:, 0:8].reshape(N_CORES, NQ, 4, 2, 4, NF, P, 2, 8, P)
    lv[:, :, :, :, :, 0:8, :] = (
        xab.transpose(0, 8, 6, 9, 1, 2, 3, 4, 5, 7)  # core,s,u,v,bq,bl,ab,c4,f,h
        .reshape(N_CORES, 8, P, P, NQ, 8, 128)
    )
    # C groups: channel 8, m=(j=batch lane, f, h)
    # axes: (core, bq, j, f, u, h, s, v)
    xc = xq[:, 8].reshape(N_CORES, NQ, 4, NF, P, 2, 8, P)
    lv[:, :, :, :, :, 8, :] = (
        xc.transpose(0, 6, 4, 7, 1, 2, 3, 5)  # core,s,u,v,bq,j,f,h
        .reshape(N_CORES, 8, P, P, NQ, 128)
    )
    return lgt.reshape(N_CORES * 128, NGROUP * 128)


def _marshal_lg_core(xq: np.ndarray, core: int) -> np.ndarray:
    """One core's slice of _marshal_lg: (128, NGROUP*128) int8."""
    sl = slice(BPC * core, BPC * (core + 1))
    lgt = np.empty((128, NQ, 9, 128), np.int8)
    lv = lgt.reshape(8, P, P, NQ, 9, 128)
    # (bq, bl, ab, c4, f, u, h, s, v) -> (s, u, v, bq, bl, ab, c4, f, h)
    xab = xq[sl, 0:8].reshape(NQ, 4, 2, 4, NF, P, 2, 8, P)
    lv[:, :, :, :, 0:8, :] = (
        xab.transpose(7, 5, 8, 0, 1, 2, 3, 4, 6).reshape(8, P, P, NQ, 8, 128)
    )
    # (bq, j, f, u, h, s, v) -> (s, u, v, bq, j, f, h)
    xc = xq[sl, 8].reshape(NQ, 4, NF, P, 2, 8, P)
    lv[:, :, :, :, 8, :] = (
        xc.transpose(5, 3, 6, 0, 1, 2, 4).reshape(8, P, P, NQ, 128)
    )
    return lgt.reshape(128, NGROUP * 128)


def _embed_table(b, channel_embed, spatial_embed, time_pos, freq_pos,
                 spatial_idx) -> np.ndarray:
    """emb[p=(c,f,t), e]: everything except the conv, f32 (2304, 192)."""
    chs = channel_embed + spatial_embed[spatial_idx] + b[None, :]  # (C, E)
    emb = (chs[:, None, None, :]
           + freq_pos[None, :, None, :]
           + time_pos[None, None, :, :])
    return np.ascontiguousarray(emb.reshape(N_PATCH, E), dtype=np.float32)


class _Exec:
    """One-time build: Bass module -> cached jitted PJRT executable."""

    def __init__(self):
        bass2jax.install_neuronx_cc_hook()
        nc = build_nc()
        self.nc = nc
        partition_name = (
            nc.partition_id_tensor.name if nc.partition_id_tensor else None
        )
        in_names: list[str] = []
        out_names: list[str] = []
        out_avals: list[jax.core.ShapedArray] = []
        for alloc in nc.m.functions[0].allocations:
            if not isinstance(alloc, mybir.MemoryLocationSet):
                continue
            name = alloc.memorylocations[0].name
            if alloc.kind == "ExternalInput":
                if name != partition_name:
                    in_names.append(name)
            elif alloc.kind == "ExternalOutput":
                out_names.append(name)
                out_avals.append(
                    jax.core.ShapedArray(
                        tuple(alloc.tensor_shape), mybir.dt.np(alloc.dtype)
                    )
                )
        n_params = len(in_names)
        n_outs = len(out_avals)
        self.in_names = list(in_names)
        self.out_names = list(out_names)
        in_names = in_names + out_names
        if partition_name is not None:
            in_names.append(partition_name)

        def _body(*args):
            operands = list(args)
            if partition_name is not None:
                operands.append(bass2jax.partition_id_tensor())
            outs = bass2jax._bass_exec_p.bind(
                *operands,
                out_avals=tuple(out_avals),
                in_names=tuple(in_names),
                out_names=tuple(out_names),
                lowering_input_output_aliases=(),
                sim_require_finite=True,
                sim_require_nnan=True,
                nc=nc,
            )
            return tuple(outs)

        devices = jax.devices()[:N_CORES]
        assert len(devices) == N_CORES, len(jax.devices())
        self.mesh = Mesh(np.asarray(devices), ("core",))
        spec = PartitionSpec("core")
        self.sharding = NamedSharding(self.mesh, spec)
        self.sharded = jax.jit(
            shard_map(
                _body,
                mesh=self.mesh,
                in_specs=(spec,) * (n_params + n_outs),
                out_specs=(spec,) * n_outs,
                check_rep=False,
            ),
            donate_argnums=tuple(range(n_params, n_params + n_outs)),
            keep_unused=True,
        )
        # donated output buffers, created on-device (no 85MB zero upload)
        self.zeros_fn = jax.jit(
            shard_map(
                lambda: (
                    jnp.zeros((BPC, N_PATCH, HB), jnp.uint8),
                    jnp.zeros((BPC, N_PATCH, LB), jnp.uint8),
                ),
                mesh=self.mesh,
                in_specs=(),
                out_specs=(spec, spec),
                check_rep=False,
            )
        )


_CACHE: dict = {}


def _get_exec() -> _Exec:
    if "exec" not in _CACHE:
        _CACHE["exec"] = _Exec()
        _CACHE["pool"] = ThreadPoolExecutor(N_CORES + 4)
    return _CACHE["exec"]


def kernel(**inputs: np.ndarray) -> np.ndarray:
    arrs = {k: np.asarray(v) for k, v in inputs.items()}
    x = arrs["x"].astype(np.float32, copy=False)
    assert x.shape == (B, C, FR, T), x.shape
    W = arrs["W"].astype(np.float32, copy=False)
    ex = _get_exec()
    pool = _CACHE["pool"]

    # input quantization xq = round(x / sx) and exact per-patch norms,
    # threaded over batch chunks (ufuncs release the GIL)
    xmax = float(max(x.max(), -x.min())) or 1.0
    sx = np.float32(xmax / 127.0)
    inv_sx = np.float32(1.0 / sx)
    xq = np.empty(x.shape, np.int8)
    pn2_parts = [0.0] * N_CORES

    def quant_chunk(i):
        sl = slice(BPC * i, BPC * (i + 1))
        xq[sl] = np.rint(x[sl] * inv_sx)

    def pn2_chunk(i):
        sl = slice(BPC * i, BPC * (i + 1))
        pn2 = np.square(x[sl]).reshape(BPC, C, NF, P, NT, P).sum(axis=(3, 5))
        pn2_parts[i] = float(pn2.max())

    qfuts = [pool.submit(quant_chunk, i) for i in range(N_CORES)]
    pfuts = [pool.submit(pn2_chunk, i) for i in range(N_CORES)]

    # 6-bit output scale via Cauchy-Schwarz: |conv| <= ||x_p|| * ||W_e||;
    # patch norms computed exactly, quantization slack of 0.5 per tap.
    wtb = W.transpose(1, 2, 0).reshape(P * P, E)  # [(u,v), e]
    wn_f = float(np.sqrt(np.square(wtb).sum(axis=0).max()))
    for f in pfuts:
        f.result()
    qn = float(np.sqrt(max(pn2_parts))) / float(sx) + 2.0  # max ||xq_p||_2
    s6 = np.float32(qn * float(sx) * wn_f / 30.9) if wn_f > 0 else np.float32(1.0)
    for _ in range(4):  # exact no-clip check on the bf16 weights actually sent
        wt16 = (wtb * (float(sx) / s6)).astype(ml_dtypes.bfloat16)
        wn = float(np.sqrt(np.square(wt16.astype(np.float32)).sum(axis=0).max()))
        if qn * wn <= 31.45:
            break
        s6 = np.float32(s6 * (qn * wn / 31.4))
    else:
        raise AssertionError("6-bit scale did not converge")
    wt_g = np.ascontiguousarray(
        np.broadcast_to(wt16, (N_CORES, 16, E)).reshape(N_CORES * 16, E)
    )
    for f in qfuts:
        f.result()

    # marshal, threaded per core into one buffer, then a single async put
    lg_full = np.empty((N_CORES * 128, NGROUP * 128), np.int8)

    def marshal_core(i):
        lg_full[128 * i : 128 * (i + 1)] = _marshal_lg_core(xq, i)

    for f in [pool.submit(marshal_core, i) for i in range(N_CORES)]:
        f.result()
    lg_dev = jax.device_put(lg_full, ex.sharding)

    emb = _embed_table(
        arrs["b"].astype(np.float32, copy=False),
        arrs["channel_embed"].astype(np.float32, copy=False),
        arrs["spatial_embed"].astype(np.float32, copy=False),
        arrs["time_pos"].astype(np.float32, copy=False),
        arrs["freq_pos"].astype(np.float32, copy=False),
        arrs["spatial_idx"],
    )
    s6f = np.float32(s6)
    emb_off = emb - np.float32(32.0) * s6f  # folds the +32 code offset

    zpair = _CACHE.pop("zeros_next", None) or ex.zeros_fn()
    hi_g, lo_g = ex.sharded(lg_dev, wt_g, *zpair)
    # donated buffers for the NEXT call, created while this call fetches
    _CACHE["zeros_next"] = ex.zeros_fn()

    # fetch shards in flight; decode with GIL-releasing ufunc arithmetic
    hi_shards = {s.index[0].start or 0: s for s in hi_g.addressable_shards}
    lo_shards = {s.index[0].start or 0: s for s in lo_g.addressable_shards}
    for s in hi_shards.values():
        s.data.copy_to_host_async()
    for s in lo_shards.values():
        s.data.copy_to_host_async()

    final = np.empty((B, N_PATCH, E), np.float32)
    CH = 8  # batches per decode task
    pool = _CACHE["pool"]

    def decode(ah, al, i0):
        hi = ah  # (CH, N_PATCH, HB) uint8, contiguous
        lo = al  # (CH, N_PATCH, LB)
        he = hi[..., 0::2]  # covers e%4 in {0,1}
        ho = hi[..., 1::2]  # covers e%4 in {2,3}
        Ub = np.empty((ah.shape[0], N_PATCH, E), np.uint8)
        t = np.right_shift(he, 4)
        np.left_shift(t, 2, out=t)
        t2 = np.right_shift(lo, 6)
        np.add(t, t2, out=t)
        Ub[..., 0::4] = t
        np.bitwise_and(he, 15, out=t)
        np.left_shift(t, 2, out=t)
        np.right_shift(lo, 4, out=t2)
        np.bitwise_and(t2, 3, out=t2)
        np.add(t, t2, out=t)
        Ub[..., 1::4] = t
        np.right_shift(ho, 4, out=t)
        np.left_shift(t, 2, out=t)
        np.right_shift(lo, 2, out=t2)
        np.bitwise_and(t2, 3, out=t2)
        np.add(t, t2, out=t)
        Ub[..., 2::4] = t
        np.bitwise_and(ho, 15, out=t)
        np.left_shift(t, 2, out=t)
        np.bitwise_and(lo, 3, out=t2)
        np.add(t, t2, out=t)
        Ub[..., 3::4] = t
        out_v = final[i0 : i0 + ah.shape[0]]
        np.multiply(Ub, s6f, out=out_v, casting="unsafe")
        np.add(out_v, emb_off[None], out=out_v)

    decode_futs = []

    def fetch(i0):
        ah = np.asarray(hi_shards[i0].data)
        al = np.asarray(lo_shards[i0].data)
        for j0 in range(0, BPC, CH):
            decode_futs.append(
                pool.submit(decode, ah[j0 : j0 + CH], al[j0 : j0 + CH], i0 + j0)
            )

    for f in [pool.submit(fetch, i0) for i0 in sorted(hi_shards)]:
        f.result()
    for f in decode_futs:
        f.result()
    return final


# revision 15
# speedup vs baseline: 1.4958x; 1.0693x over previous
"""Trainium2 Bass kernel for ChannelPatchEmbedding (dense_cnn).

Computes, for x:(B,C,64,64):
    out[b, c*256 + f*16 + t0, e] =
        sum_{u,v} x[b,c,4f+u,4t0+v] * W[e,u,v] + bias[e]
        + channel_embed[c,e] + spatial_embed[spatial_idx[c],e]
        + freq_pos[f,e] + time_pos[t0,e]

Sharding: pure data parallel over the batch dim across 8 NeuronCores.

End-to-end wall time is dominated by the axon tunnel, whose ~50MB/s
capacity is shared between directions, so the implementation minimizes
total bytes moved and per-call overhead:

  - x ships as int8 (xq = round(x/sx), 1.13MB/core); the device widens
    it to bf16. The conv weights ship as a 6KB bf16 table holding
    W^T * sx / s6 and are placed on the s-block-diagonal on device, so
    one matmul computes 8 patch-octets and PSUM holds conv/s6 with
    |psum| <= 31.45 guaranteed by a Cauchy-Schwarz bound on the host
    (max patch 2-norm x max filter 2-norm): the 6-bit quantization
    below never clips. Quantization, patch norms, and marshaling are
    threaded over batch chunks; the lg upload is a single async
    device_put issued right after marshaling.
  - Per 1024-patch group: 3 matmuls (N=512, one PSUM bank each), then
    DVE evicts U = uint8(psum + 32) (6-bit codes 1..63) and packs U
    into two separate streams: 4-bit nibbles (96B/patch) and 2-bit
    fields (48B/patch): 0.75 bytes/value, 84.9MB total, max quant
    error s6/2 -> rel err ~3e-3, 6x under tolerance.
  - Stores are contiguous DMAs alternating the two HWDGE rings.
  - The jitted PJRT executable is built once and cached; donated output
    buffers are created on-device (no zero upload); shards are fetched
    with copy_to_host_async and decoded into the final f32 buffer by a
    thread pool while later shards are still in flight. The decode uses
    only ufunc arithmetic (shifts/masks), which releases the GIL, so it
    genuinely parallelizes and hides under the transfer.
"""

from concurrent.futures import ThreadPoolExecutor

import numpy as np
import ml_dtypes
import jax
import jax.numpy as jnp
from jax.experimental.shard_map import shard_map
from jax.sharding import Mesh, NamedSharding, PartitionSpec

import concourse.bass as bass
import concourse.mybir as mybir
from concourse import bass2jax
from concourse.tile import TileContext
from concourse.vector_clock import ScopedClock

f32 = mybir.dt.float32
bf16 = mybir.dt.bfloat16
i8 = mybir.dt.int8
u8 = mybir.dt.uint8
ALU = mybir.AluOpType

B, C, FR, T = 256, 9, 64, 64
P, E = 4, 192
NF = NT = 16
N_PATCH = C * NF * NT  # 2304
N_CORES = 8
BPC = B // N_CORES  # 32
NQ = BPC // 4  # 8 batch-quads per core
NGROUP = 72  # per core: 8 quads x (4 batches x {A,B} + C)
W8 = 8 * E  # 1536 psum cols per group
HB = 96  # hi-nibble bytes per patch
LB = 48  # lo-2bit bytes per patch


class _TC(TileContext):
    """TileContext whose kernel-tail drain never carries more than one
    sync-wait: the walrus build in this container rejects multi-wait CTRL
    instructions, and the stock tail Drain aggregates every residual
    proc wait onto itself. Spread them across single-wait SP nops."""

    def _drain_and_barrier(self, tick_clock, wait_clock):
        probe = self.nc.sync.nop()
        wait_clock.add_sem_waits(
            probe.ins, ScopedClock({None: tick_clock.global_clock})
        )
        si = probe.ins.sync_info
        waits = list(si.on_wait) if si is not None and si.on_wait else []
        if len(waits) > 1:
            si.on_wait = waits[:1]
            for w in waits[1:]:
                n2 = self.nc.sync.nop()
                si2 = n2.ins.sync_info
                if si2 is None:
                    n2.ins.sync_info = mybir.SyncInfo(on_wait=[w], on_update=[])
                else:
                    si2.on_wait = [w]
        self.nc.sync.drain()
        self.nc.all_engine_barrier()
        popped = self.nc._tile_sem_poison_stack.pop()
        assert popped is self._sem_poison
        self.nc.clear_and_free_semaphores(list(self.sems.allocated().values()))
        self.nc.all_engine_barrier()


def _split_multi_waits(nc: bass.Bass, max_waits: int = 1) -> None:
    """This container's walrus rejects instructions carrying more than one
    sync-wait. Move excess waits onto same-engine NoOps inserted right
    before the instruction (equivalent semantics: the sequencer blocks on
    each in turn)."""
    for fn in nc.m.functions:
        for blk in fn.blocks:
            out, changed = [], False
            for inst in list(blk.instructions):
                si = inst.sync_info
                if si is not None and si.on_wait and len(si.on_wait) > max_waits:
                    waits = list(si.on_wait)
                    for i, w in enumerate(waits[:-max_waits]):
                        out.append(
                            mybir.InstNoOp(
                                name=f"{inst.name}-wsplit{i}",
                                engine=inst.engine,
                                sync_info=mybir.SyncInfo(
                                    on_wait=[w], on_update=[]
                                ),
                            )
                        )
                    si.on_wait = waits[-max_waits:]
                    changed = True
                out.append(inst)
            if changed:
                blk.instructions = out


def build_nc() -> bass.Bass:
    nc = bass.Bass(trn_type="TRN2", debug=False)

    lgq = nc.dram_tensor("lgq", [128, NGROUP * 128], i8, kind="ExternalInput")
    wt_d = nc.dram_tensor("wt", [16, E], bf16, kind="ExternalInput")
    outh = nc.dram_tensor("outh", [BPC, N_PATCH, HB], u8, kind="ExternalOutput")
    outl = nc.dram_tensor("outl", [BPC, N_PATCH, LB], u8, kind="ExternalOutput")

    outha = outh.ap()
    outla = outl.ap()

    with _TC(nc) as tc:
        with (
            tc.tile_pool(name="const", bufs=1) as cp,
            tc.tile_pool(name="outph", bufs=8) as outph,
            tc.tile_pool(name="outpl", bufs=8) as outpl,
            tc.tile_pool(name="outch", bufs=3) as outch,
            tc.tile_pool(name="outcl", bufs=3) as outcl,
            tc.tile_pool(name="upool", bufs=3) as upool,
            tc.tile_pool(name="vpool", bufs=4) as vpool,
            tc.tile_pool(name="mpool", bufs=2) as mpool,
            tc.tile_pool(name="psum", bufs=2, space="PSUM") as psp,
        ):
            # block-diagonal weights, built on device from the 6KB table
            rhs_sb = cp.tile([128, W8], bf16, name="rhs_sb")
            wt_sb = cp.tile([16, E], bf16, name="wt_sb")
            nc.vector.memset(rhs_sb[:, :], 0)
            nc.gpsimd.dma_start(out=wt_sb[:, :], in_=wt_d.ap())
            for s in range(8):
                nc.gpsimd.dma_start(
                    out=rhs_sb[16 * s : 16 * s + 16, E * s : E * (s + 1)],
                    in_=wt_sb[:, :],
                )
            # lgq in sixths, alternating rings; DVE widens int8 -> bf16
            lgts = []
            LW = NGROUP * 128 // 6
            for li in range(6):
                tq = cp.tile([128, LW], i8, name=f"lgq{li}")
                t = cp.tile([128, LW], bf16, name=f"lgt{li}")
                (nc.sync if li % 2 == 0 else nc.scalar).dma_start(
                    out=tq[:, :], in_=lgq.ap()[:, li * LW : (li + 1) * LW]
                )
                nc.vector.tensor_copy(out=t[:, :], in_=tq[:, :])
                lgts.append(t)

            def mm_group(g: int, oth, otl, hcol0: int, lcol0: int):
                """3 matmuls -> psum = conv/s6; evict U = uint8(psum+32)
                (values 1..63); pack nibbles into oth, 2-bit fields into
                otl."""
                gl = g % 12
                lhs = lgts[g // 12][:, 128 * gl : 128 * (gl + 1)]
                ps = psp.tile([128, W8], f32, name="ps")
                for p3 in range(3):
                    nc.tensor.matmul(
                        out=ps[:, 512 * p3 : 512 * (p3 + 1)],
                        lhsT=lhs,
                        rhs=rhs_sb[:, 512 * p3 : 512 * (p3 + 1)],
                        start=True,
                        stop=True,
                    )
                U = upool.tile([128, W8], u8, name="U")
                nc.vector.tensor_scalar(
                    out=U[:, :], in0=ps[:, :], scalar1=32.0, scalar2=None,
                    op0=ALU.add,
                )
                V = vpool.tile([128, W8], u8, name="V")  # U >> 2 (4-bit)
                nc.vector.tensor_scalar(
                    out=V[:, :], in0=U[:, :], scalar1=2, scalar2=None,
                    op0=ALU.logical_shift_right,
                )
                L = vpool.tile([128, W8], u8, name="L")  # U & 3 (2-bit)
                nc.vector.tensor_scalar(
                    out=L[:, :], in0=U[:, :], scalar1=3, scalar2=None,
                    op0=ALU.bitwise_and,
                )
                m1 = mpool.tile([128, 2 * 384], u8, name="m1")
                for s in range(8):
                    e0 = E * s
                    # hi: nibble pairs -> bytes (even e in high nibble)
                    nc.vector.scalar_tensor_tensor(
                        out=oth[:, hcol0 + HB * s : hcol0 + HB * (s + 1)],
                        in0=V[:, e0 : e0 + E : 2], scalar=16.0,
                        in1=V[:, e0 + 1 : e0 + E : 2],
                        op0=ALU.mult, op1=ALU.add,
                    )
                    # lo: 2-bit quads -> bytes (e%4==0 in bits 7:6, ==3 in 1:0)
                    nc.vector.scalar_tensor_tensor(
                        out=m1[:, 48 * s : 48 * s + 48],
                        in0=L[:, e0 : e0 + E : 4], scalar=4.0,
                        in1=L[:, e0 + 1 : e0 + E : 4],
                        op0=ALU.mult, op1=ALU.add,
                    )
                    nc.vector.scalar_tensor_tensor(
                        out=m1[:, 384 + 48 * s : 384 + 48 * s + 48],
                        in0=L[:, e0 + 2 : e0 + E : 4], scalar=4.0,
                        in1=L[:, e0 + 3 : e0 + E : 4],
                        op0=ALU.mult, op1=ALU.add,
                    )
                for s in range(8):
                    nc.vector.scalar_tensor_tensor(
                        out=otl[:, lcol0 + LB * s : lcol0 + LB * (s + 1)],
                        in0=m1[:, 48 * s : 48 * s + 48], scalar=16.0,
                        in1=m1[:, 384 + 48 * s : 384 + 48 * s + 48],
                        op0=ALU.mult, op1=ALU.add,
                    )

            g = 0
            store_engs = (nc.sync, nc.scalar)
            dma_flip = 0
            for bq in range(NQ):
                for bl in range(4):
                    b = 4 * bq + bl
                    oth = outph.tile([128, 2 * 8 * HB], u8, name="oth")
                    otl = outpl.tile([128, 2 * 8 * LB], u8, name="otl")
                    mm_group(g, oth, otl, 0, 0)  # A: channels 0-3
                    g += 1
                    mm_group(g, oth, otl, 8 * HB, 8 * LB)  # B: channels 4-7
                    g += 1
                    dsth = outha[b, 0 : 2 * 1024, :].rearrange(
                        "(blk m s) j -> m blk (s j)", blk=2, s=8
                    )
                    store_engs[dma_flip % 2].dma_start(out=dsth, in_=oth[:, :])
                    dma_flip += 1
                    dstl = outla[b, 0 : 2 * 1024, :].rearrange(
                        "(blk m s) j -> m blk (s j)", blk=2, s=8
                    )
                    store_engs[dma_flip % 2].dma_start(out=dstl, in_=otl[:, :])
                    dma_flip += 1
                # C: channel 8 of the 4 batches in this quad
                oth = outch.tile([128, 8 * HB], u8, name="otch")
                otl = outcl.tile([128, 8 * LB], u8, name="otcl")
                mm_group(g, oth, otl, 0, 0)
                g += 1
                dsth = outha[4 * bq : 4 * bq + 4, 8 * 256 : 9 * 256, :].rearrange(
                    "j (r s) e -> j r (s e)", s=8
                )
                store_engs[dma_flip % 2].dma_start(out=dsth, in_=oth[:, :])
                dma_flip += 1
                dstl = outla[4 * bq : 4 * bq + 4, 8 * 256 : 9 * 256, :].rearrange(
                    "j (r s) e -> j r (s e)", s=8
                )
                store_engs[dma_flip % 2].dma_start(out=dstl, in_=otl[:, :])
                dma_flip += 1
            assert g == NGROUP

    _split_multi_waits(nc)
    return nc


def _marshal_lg(xq: np.ndarray) -> np.ndarray:
    """Build the global lhsT input (N_CORES*128, NGROUP*128) int8 from the
    int8-quantized x.

    Layout per core: lg[k, g*128 + m], k=(s,u,v), groups ordered
    [A(b0),B(b0),...,A(b3),B(b3),C] per batch-quad; m=(c%4|j, f, h).
    x[b, c, 4f+u, 32h+4s+v]."""
    lgt = np.empty((N_CORES, 128, NQ, 9, 128), np.int8)
    lv = lgt.reshape(N_CORES, 8, P, P, NQ, 9, 128)  # k -> (s, u, v)
    # A/B groups: channels 0-7
    # axes: (core, bq, bl, ab, c4, f, u, h, s, v)
    xab = xq[:, 0:8].reshape(N_CORES, NQ, 4, 2, 4, NF, P, 2, 8, P)
    lv[:, :, :, :, :, 0:8, :] = (
        xab.transpose(0, 8, 6, 9, 1, 2, 3, 4, 5, 7)  # core,s,u,v,bq,bl,ab,c4,f,h
        .reshape(N_CORES, 8, P, P, NQ, 8, 128)
    )
    # C groups: channel 8, m=(j=batch lane, f, h)
    # axes: (core, bq, j, f, u, h, s, v)
    xc = xq[:, 8].reshape(N_CORES, NQ, 4, NF, P, 2, 8, P)
    lv[:, :, :, :, :, 8, :] = (
        xc.transpose(0, 6, 4, 7, 1, 2, 3, 5)  # core,s,u,v,bq,j,f,h
        .reshape(N_CORES, 8, P, P, NQ, 128)
    )
    return lgt.reshape(N_CORES * 128, NGROUP * 128)


def _marshal_lg_core(xq: np.ndarray, core: int) -> np.ndarray:
    """One core's slice of _marshal_lg: (128, NGROUP*128) int8."""
    sl = slice(BPC * core, BPC * (core + 1))
    lgt = np.empty((128, NQ, 9, 128), np.int8)
    lv = lgt.reshape(8, P, P, NQ, 9, 128)
    # (bq, bl, ab, c4, f, u, h, s, v) -> (s, u, v, bq, bl, ab, c4, f, h)
    xab = xq[sl, 0:8].reshape(NQ, 4, 2, 4, NF, P, 2, 8, P)
    lv[:, :, :, :, 0:8, :] = (
        xab.transpose(7, 5, 8, 0, 1, 2, 3, 4, 6).reshape(8, P, P, NQ, 8, 128)
    )
    # (bq, j, f, u, h, s, v) -> (s, u, v, bq, j, f, h)
    xc = xq[sl, 8].reshape(NQ, 4, NF, P, 2, 8, P)
    lv[:, :, :, :, 8, :] = (
        xc.transpose(5, 3, 6, 0, 1, 2, 4).reshape(8, P, P, NQ, 128)
    )
    return lgt.reshape(128, NGROUP * 128)


def _embed_table(b, channel_embed, spatial_embed, time_pos, freq_pos,
                 spatial_idx) -> np.ndarray:
    """emb[p=(c,f,t), e]: everything except the conv, f32 (2304, 192)."""
    chs = channel_embed + spatial_embed[spatial_idx] + b[None, :]  # (C, E)
    emb = (chs[:, None, None, :]
           + freq_pos[None, :, None, :]
           + time_pos[None, None, :, :])
    return np.ascontiguousarray(emb.reshape(N_PATCH, E), dtype=np.float32)


class _Exec:
    """One-time build: Bass module -> cached jitted PJRT executable."""

    def __init__(self):
        bass2jax.install_neuronx_cc_hook()
        nc = build_nc()
        self.nc = nc
        partition_name = (
            nc.partition_id_tensor.name if nc.partition_id_tensor else None
        )
        in_names: list[str] = []
        out_names: list[str] = []
        out_avals: list[jax.core.ShapedArray] = []
        for alloc in nc.m.functions[0].allocations:
            if not isinstance(alloc, mybir.MemoryLocationSet):
                continue
            name = alloc.memorylocations[0].name
            if alloc.kind == "ExternalInput":
                if name != partition_name:
                    in_names.append(name)
            elif alloc.kind == "ExternalOutput":
                out_names.append(name)
                out_avals.append(
                    jax.core.ShapedArray(
                        tuple(alloc.tensor_shape), mybir.dt.np(alloc.dtype)
                    )
                )
        n_params = len(in_names)
        n_outs = len(out_avals)
        self.in_names = list(in_names)
        self.out_names = list(out_names)
        in_names = in_names + out_names
        if partition_name is not None:
            in_names.append(partition_name)

        def _body(*args):
            operands = list(args)
            if partition_name is not None:
                operands.append(bass2jax.partition_id_tensor())
            outs = bass2jax._bass_exec_p.bind(
                *operands,
                out_avals=tuple(out_avals),
                in_names=tuple(in_names),
                out_names=tuple(out_names),
                lowering_input_output_aliases=(),
                sim_require_finite=True,
                sim_require_nnan=True,
                nc=nc,
            )
            return tuple(outs)

        devices = jax.devices()[:N_CORES]
        assert len(devices) == N_CORES, len(jax.devices())
        self.mesh = Mesh(np.asarray(devices), ("core",))
        spec = PartitionSpec("core")
        self.sharding = NamedSharding(self.mesh, spec)
        self.sharded = jax.jit(
            shard_map(
                _body,
                mesh=self.mesh,
                in_specs=(spec,) * (n_params + n_outs),
                out_specs=(spec,) * n_outs,
                check_rep=False,
            ),
            donate_argnums=tuple(range(n_params, n_params + n_outs)),
            keep_unused=True,
        )
        # donated output buffers, created on-device (no 85MB zero upload)
        self.zeros_fn = jax.jit(
            shard_map(
                lambda: (
                    jnp.zeros((BPC, N_PATCH, HB), jnp.uint8),
                    jnp.zeros((BPC, N_PATCH, LB), jnp.uint8),
                ),
                mesh=self.mesh,
                in_specs=(),
                out_specs=(spec, spec),
                check_rep=False,
            )
        )


_CACHE: dict = {}


def _get_exec() -> _Exec:
    if "exec" not in _CACHE:
        _CACHE["exec"] = _Exec()
        _CACHE["pool"] = ThreadPoolExecutor(N_CORES + 4)
    return _CACHE["exec"]


def kernel(**inputs: np.ndarray) -> np.ndarray:
    arrs = {k: np.asarray(v) for k, v in inputs.items()}
    x = arrs["x"].astype(np.float32, copy=False)
    assert x.shape == (B, C, FR, T), x.shape
    W = arrs["W"].astype(np.float32, copy=False)
    ex = _get_exec()
    pool = _CACHE["pool"]

    # input quantization xq = round(x / sx) and exact per-patch norms,
    # threaded over batch chunks (ufuncs release the GIL)
    xmax = float(max(x.max(), -x.min())) or 1.0
    sx = np.float32(xmax / 127.0)
    inv_sx = np.float32(1.0 / sx)
    xq = np.empty(x.shape, np.int8)
    pn2_parts = [0.0] * N_CORES

    def quant_chunk(i):
        sl = slice(BPC * i, BPC * (i + 1))
        xq[sl] = np.rint(x[sl] * inv_sx)

    def pn2_chunk(i):
        sl = slice(BPC * i, BPC * (i + 1))
        pn2 = np.square(x[sl]).reshape(BPC, C, NF, P, NT, P).sum(axis=(3, 5))
        pn2_parts[i] = float(pn2.max())

    qfuts = [pool.submit(quant_chunk, i) for i in range(N_CORES)]
    pfuts = [pool.submit(pn2_chunk, i) for i in range(N_CORES)]

    # 6-bit output scale via Cauchy-Schwarz: |conv| <= ||x_p|| * ||W_e||;
    # patch norms computed exactly, quantization slack of 0.5 per tap.
    wtb = W.transpose(1, 2, 0).reshape(P * P, E)  # [(u,v), e]
    wn_f = float(np.sqrt(np.square(wtb).sum(axis=0).max()))
    for f in pfuts:
        f.result()
    qn = float(np.sqrt(max(pn2_parts))) / float(sx) + 2.0  # max ||xq_p||_2
    s6 = np.float32(qn * float(sx) * wn_f / 30.9) if wn_f > 0 else np.float32(1.0)
    for _ in range(4):  # exact no-clip check on the bf16 weights actually sent
        wt16 = (wtb * (float(sx) / s6)).astype(ml_dtypes.bfloat16)
        wn = float(np.sqrt(np.square(wt16.astype(np.float32)).sum(axis=0).max()))
        if qn * wn <= 31.45:
            break
        s6 = np.float32(s6 * (qn * wn / 31.4))
    else:
        raise AssertionError("6-bit scale did not converge")
    wt_g = np.ascontiguousarray(
        np.broadcast_to(wt16, (N_CORES, 16, E)).reshape(N_CORES * 16, E)
    )
    for f in qfuts:
        f.result()

    # marshal, threaded per core into one buffer, then a single async put
    lg_full = np.empty((N_CORES * 128, NGROUP * 128), np.int8)

    def marshal_core(i):
        lg_full[128 * i : 128 * (i + 1)] = _marshal_lg_core(xq, i)

    for f in [pool.submit(marshal_core, i) for i in range(N_CORES)]:
        f.result()
    lg_dev = jax.device_put(lg_full, ex.sharding)

    emb = _embed_table(
        arrs["b"].astype(np.float32, copy=False),
        arrs["channel_embed"].astype(np.float32, copy=False),
        arrs["spatial_embed"].astype(np.float32, copy=False),
        arrs["time_pos"].astype(np.float32, copy=False),
        arrs["freq_pos"].astype(np.float32, copy=False),
        arrs["spatial_idx"],
    )
    s6f = np.float32(s6)
    emb_off = emb - np.float32(32.0) * s6f  # folds the +32 code offset

    zpair = _CACHE.pop("zeros_next", None) or ex.zeros_fn()
    hi_g, lo_g = ex.sharded(lg_dev, wt_g, *zpair)
    # donated buffers for the NEXT call, created while this call fetches
    _CACHE["zeros_next"] = ex.zeros_fn()

    # fetch shards in flight; decode with GIL-releasing ufunc arithmetic
    hi_shards = {s.index[0].start or 0: s for s in hi_g.addressable_shards}
    lo_shards = {s.index[0].start or 0: s for s in lo_g.addressable_shards}
    for s in hi_shards.values():
        s.data.copy_to_host_async()
    for s in lo_shards.values():
        s.data.copy_to_host_async()

    final = np.empty((B, N_PATCH, E), np.float32)
    CH = 8  # batches per decode task
    pool = _CACHE["pool"]

    def decode(ah, al, i0):
        hi = ah  # (CH, N_PATCH, HB) uint8, contiguous
        lo = al  # (CH, N_PATCH, LB)
        he = hi[..., 0::2]  # covers e%4 in {0,1}
        ho = hi[..., 1::2]  # covers e%4 in {2,3}
        Ub = np.empty((ah.shape[0], N_PATCH, E), np.uint8)
        t = np.right_shift(he, 4)
        np.left_shift(t, 2, out=t)
        t2 = np.right_shift(lo, 6)
        np.add(t, t2, out=t)
        Ub[..., 0::4] = t
        np.bitwise_and(he, 15, out=t)
        np.left_shift(t, 2, out=t)
        np.right_shift(lo, 4, out=t2)
        np.bitwise_and(t2, 3, out=t2)
        np.add(t, t2, out=t)
        Ub[..., 1::4] = t
        np.right_shift(ho, 4, out=t)
        np.left_shift(t, 2, out=t)
        np.right_shift(lo, 2, out=t2)
        np.bitwise_and(t2, 3, out=t2)
        np.add(t, t2, out=t)
        Ub[..., 2::4] = t
        np.bitwise_and(ho, 15, out=t)
        np.left_shift(t, 2, out=t)
        np.bitwise_and(lo, 3, out=t2)
        np.add(t, t2, out=t)
        Ub[..., 3::4] = t
        out_v = final[i0 : i0 + ah.shape[0]]
        np.multiply(Ub, s6f, out=out_v, casting="unsafe")
        np.add(out_v, emb_off[None], out=out_v)

    decode_futs = []

    def fetch(i0):
        ah = np.asarray(hi_shards[i0].data)
        al = np.asarray(lo_shards[i0].data)
        for j0 in range(0, BPC, CH):
            decode_futs.append(
                pool.submit(decode, ah[j0 : j0 + CH], al[j0 : j0 + CH], i0 + j0)
            )

    for f in [pool.submit(fetch, i0) for i0 in sorted(hi_shards)]:
        f.result()
    for f in decode_futs:
        f.result()
    return final


# revision 17
# speedup vs baseline: 1.6912x; 1.1307x over previous
"""Trainium2 Bass kernel for ChannelPatchEmbedding (dense_cnn).

Computes, for x:(B,C,64,64):
    out[b, c*256 + f*16 + t0, e] =
        sum_{u,v} x[b,c,4f+u,4t0+v] * W[e,u,v] + bias[e]
        + channel_embed[c,e] + spatial_embed[spatial_idx[c],e]
        + freq_pos[f,e] + time_pos[t0,e]

Sharding: pure data parallel over the batch dim across 8 NeuronCores.

End-to-end wall time is dominated by the axon tunnel, whose ~50MB/s
capacity is shared between directions, so the implementation minimizes
total bytes moved and per-call overhead:

  - x ships as int8 (xq = round(x/sx), 1.13MB/core); the device widens
    it to bf16. The conv weights ship as a 6KB bf16 table holding
    W^T * sx / s6 and are placed on the s-block-diagonal on device, so
    one matmul computes 8 patch-octets and PSUM holds conv/s6 with
    |psum| <= 31.45 guaranteed by a Cauchy-Schwarz bound on the host
    (max patch 2-norm x max filter 2-norm): the 6-bit quantization
    below never clips. Quantization, patch norms, and marshaling are
    threaded over batch chunks; the lg upload is a single async
    device_put issued right after marshaling.
  - Per 1024-patch group: 3 matmuls (N=512, one PSUM bank each), then
    DVE evicts U = uint8(psum + 32) (6-bit codes 1..63) and packs U
    into two separate streams: 4-bit nibbles (96B/patch) and 2-bit
    fields (48B/patch): 0.75 bytes/value, 84.9MB total, max quant
    error s6/2 -> rel err ~3e-3, 6x under tolerance.
  - Stores are contiguous DMAs alternating the two HWDGE rings.
  - The jitted PJRT executable is built once and cached; donated output
    buffers are created on-device (no zero upload); shards are fetched
    with copy_to_host_async and decoded into the final f32 buffer by a
    thread pool while later shards are still in flight. The decode uses
    only ufunc arithmetic (shifts/masks), which releases the GIL, so it
    genuinely parallelizes and hides under the transfer.
"""

from concurrent.futures import ThreadPoolExecutor

import numpy as np
import ml_dtypes
import jax
import jax.numpy as jnp
from jax.experimental.shard_map import shard_map
from jax.sharding import Mesh, NamedSharding, PartitionSpec

import concourse.bass as bass
import concourse.mybir as mybir
from concourse import bass2jax
from concourse.tile import TileContext
from concourse.vector_clock import ScopedClock

f32 = mybir.dt.float32
bf16 = mybir.dt.bfloat16
i8 = mybir.dt.int8
u8 = mybir.dt.uint8
ALU = mybir.AluOpType

B, C, FR, T = 256, 9, 64, 64
P, E = 4, 192
NF = NT = 16
N_PATCH = C * NF * NT  # 2304
N_CORES = 8
BPC = B // N_CORES  # 32
NQ = BPC // 4  # 8 batch-quads per core
NGROUP = 72  # per core: 8 quads x (4 batches x {A,B} + C)
W8 = 8 * E  # 1536 psum cols per group
HB = 96  # hi-nibble bytes per patch
LB = 48  # lo-2bit bytes per patch


class _TC(TileContext):
    """TileContext whose kernel-tail drain never carries more than one
    sync-wait: the walrus build in this container rejects multi-wait CTRL
    instructions, and the stock tail Drain aggregates every residual
    proc wait onto itself. Spread them across single-wait SP nops."""

    def _drain_and_barrier(self, tick_clock, wait_clock):
        probe = self.nc.sync.nop()
        wait_clock.add_sem_waits(
            probe.ins, ScopedClock({None: tick_clock.global_clock})
        )
        si = probe.ins.sync_info
        waits = list(si.on_wait) if si is not None and si.on_wait else []
        if len(waits) > 1:
            si.on_wait = waits[:1]
            for w in waits[1:]:
                n2 = self.nc.sync.nop()
                si2 = n2.ins.sync_info
                if si2 is None:
                    n2.ins.sync_info = mybir.SyncInfo(on_wait=[w], on_update=[])
                else:
                    si2.on_wait = [w]
        self.nc.sync.drain()
        self.nc.all_engine_barrier()
        popped = self.nc._tile_sem_poison_stack.pop()
        assert popped is self._sem_poison
        self.nc.clear_and_free_semaphores(list(self.sems.allocated().values()))
        self.nc.all_engine_barrier()


def _split_multi_waits(nc: bass.Bass, max_waits: int = 1) -> None:
    """This container's walrus rejects instructions carrying more than one
    sync-wait. Move excess waits onto same-engine NoOps inserted right
    before the instruction (equivalent semantics: the sequencer blocks on
    each in turn)."""
    for fn in nc.m.functions:
        for blk in fn.blocks:
            out, changed = [], False
            for inst in list(blk.instructions):
                si = inst.sync_info
                if si is not None and si.on_wait and len(si.on_wait) > max_waits:
                    waits = list(si.on_wait)
                    for i, w in enumerate(waits[:-max_waits]):
                        out.append(
                            mybir.InstNoOp(
                                name=f"{inst.name}-wsplit{i}",
                                engine=inst.engine,
                                sync_info=mybir.SyncInfo(
                                    on_wait=[w], on_update=[]
                                ),
                            )
                        )
                    si.on_wait = waits[-max_waits:]
                    changed = True
                out.append(inst)
            if changed:
                blk.instructions = out


def build_nc() -> bass.Bass:
    nc = bass.Bass(trn_type="TRN2", debug=False)

    lgq = nc.dram_tensor("lgq", [128, NGROUP * 128], i8, kind="ExternalInput")
    wt_d = nc.dram_tensor("wt", [16, E], bf16, kind="ExternalInput")
    outh = nc.dram_tensor("outh", [BPC, N_PATCH, HB], u8, kind="ExternalOutput")
    outl = nc.dram_tensor("outl", [BPC, N_PATCH, LB], u8, kind="ExternalOutput")

    outha = outh.ap()
    outla = outl.ap()

    with _TC(nc) as tc:
        with (
            tc.tile_pool(name="const", bufs=1) as cp,
            tc.tile_pool(name="outph", bufs=8) as outph,
            tc.tile_pool(name="outpl", bufs=8) as outpl,
            tc.tile_pool(name="outch", bufs=3) as outch,
            tc.tile_pool(name="outcl", bufs=3) as outcl,
            tc.tile_pool(name="upool", bufs=3) as upool,
            tc.tile_pool(name="vpool", bufs=4) as vpool,
            tc.tile_pool(name="mpool", bufs=2) as mpool,
            tc.tile_pool(name="psum", bufs=2, space="PSUM") as psp,
        ):
            # block-diagonal weights, built on device from the 6KB table
            rhs_sb = cp.tile([128, W8], bf16, name="rhs_sb")
            wt_sb = cp.tile([16, E], bf16, name="wt_sb")
            nc.vector.memset(rhs_sb[:, :], 0)
            nc.gpsimd.dma_start(out=wt_sb[:, :], in_=wt_d.ap())
            for s in range(8):
                nc.gpsimd.dma_start(
                    out=rhs_sb[16 * s : 16 * s + 16, E * s : E * (s + 1)],
                    in_=wt_sb[:, :],
                )
            # lgq in sixths, alternating rings; DVE widens int8 -> bf16
            lgts = []
            LW = NGROUP * 128 // 6
            for li in range(6):
                tq = cp.tile([128, LW], i8, name=f"lgq{li}")
                t = cp.tile([128, LW], bf16, name=f"lgt{li}")
                (nc.sync if li % 2 == 0 else nc.scalar).dma_start(
                    out=tq[:, :], in_=lgq.ap()[:, li * LW : (li + 1) * LW]
                )
                nc.vector.tensor_copy(out=t[:, :], in_=tq[:, :])
                lgts.append(t)

            def mm_group(g: int, oth, otl, hcol0: int, lcol0: int):
                """3 matmuls -> psum = conv/s6; evict U = uint8(psum+32)
                (values 1..63); pack nibbles into oth, 2-bit fields into
                otl."""
                gl = g % 12
                lhs = lgts[g // 12][:, 128 * gl : 128 * (gl + 1)]
                ps = psp.tile([128, W8], f32, name="ps")
                for p3 in range(3):
                    nc.tensor.matmul(
                        out=ps[:, 512 * p3 : 512 * (p3 + 1)],
                        lhsT=lhs,
                        rhs=rhs_sb[:, 512 * p3 : 512 * (p3 + 1)],
                        start=True,
                        stop=True,
                    )
                U = upool.tile([128, W8], u8, name="U")
                nc.vector.tensor_scalar(
                    out=U[:, :], in0=ps[:, :], scalar1=32.0, scalar2=None,
                    op0=ALU.add,
                )
                V = vpool.tile([128, W8], u8, name="V")  # U >> 2 (4-bit)
                nc.vector.tensor_scalar(
                    out=V[:, :], in0=U[:, :], scalar1=2, scalar2=None,
                    op0=ALU.logical_shift_right,
                )
                L = vpool.tile([128, W8], u8, name="L")  # U & 3 (2-bit)
                nc.vector.tensor_scalar(
                    out=L[:, :], in0=U[:, :], scalar1=3, scalar2=None,
                    op0=ALU.bitwise_and,
                )
                m1 = mpool.tile([128, 2 * 384], u8, name="m1")
                for s in range(8):
                    e0 = E * s
                    # hi: nibble pairs -> bytes (even e in high nibble)
                    nc.vector.scalar_tensor_tensor(
                        out=oth[:, hcol0 + HB * s : hcol0 + HB * (s + 1)],
                        in0=V[:, e0 : e0 + E : 2], scalar=16.0,
                        in1=V[:, e0 + 1 : e0 + E : 2],
                        op0=ALU.mult, op1=ALU.add,
                    )
                    # lo: 2-bit quads -> bytes (e%4==0 in bits 7:6, ==3 in 1:0)
                    nc.vector.scalar_tensor_tensor(
                        out=m1[:, 48 * s : 48 * s + 48],
                        in0=L[:, e0 : e0 + E : 4], scalar=4.0,
                        in1=L[:, e0 + 1 : e0 + E : 4],
                        op0=ALU.mult, op1=ALU.add,
                    )
                    nc.vector.scalar_tensor_tensor(
                        out=m1[:, 384 + 48 * s : 384 + 48 * s + 48],
                        in0=L[:, e0 + 2 : e0 + E : 4], scalar=4.0,
                        in1=L[:, e0 + 3 : e0 + E : 4],
                        op0=ALU.mult, op1=ALU.add,
                    )
                for s in range(8):
                    nc.vector.scalar_tensor_tensor(
                        out=otl[:, lcol0 + LB * s : lcol0 + LB * (s + 1)],
                        in0=m1[:, 48 * s : 48 * s + 48], scalar=16.0,
                        in1=m1[:, 384 + 48 * s : 384 + 48 * s + 48],
                        op0=ALU.mult, op1=ALU.add,
                    )

            g = 0
            store_engs = (nc.sync, nc.scalar)
            dma_flip = 0
            for bq in range(NQ):
                for bl in range(4):
                    b = 4 * bq + bl
                    oth = outph.tile([128, 2 * 8 * HB], u8, name="oth")
                    otl = outpl.tile([128, 2 * 8 * LB], u8, name="otl")
                    mm_group(g, oth, otl, 0, 0)  # A: channels 0-3
                    g += 1
                    mm_group(g, oth, otl, 8 * HB, 8 * LB)  # B: channels 4-7
                    g += 1
                    dsth = outha[b, 0 : 2 * 1024, :].rearrange(
                        "(blk m s) j -> m blk (s j)", blk=2, s=8
                    )
                    store_engs[dma_flip % 2].dma_start(out=dsth, in_=oth[:, :])
                    dma_flip += 1
                    dstl = outla[b, 0 : 2 * 1024, :].rearrange(
                        "(blk m s) j -> m blk (s j)", blk=2, s=8
                    )
                    store_engs[dma_flip % 2].dma_start(out=dstl, in_=otl[:, :])
                    dma_flip += 1
                # C: channel 8 of the 4 batches in this quad
                oth = outch.tile([128, 8 * HB], u8, name="otch")
                otl = outcl.tile([128, 8 * LB], u8, name="otcl")
                mm_group(g, oth, otl, 0, 0)
                g += 1
                dsth = outha[4 * bq : 4 * bq + 4, 8 * 256 : 9 * 256, :].rearrange(
                    "j (r s) e -> j r (s e)", s=8
                )
                store_engs[dma_flip % 2].dma_start(out=dsth, in_=oth[:, :])
                dma_flip += 1
                dstl = outla[4 * bq : 4 * bq + 4, 8 * 256 : 9 * 256, :].rearrange(
                    "j (r s) e -> j r (s e)", s=8
                )
                store_engs[dma_flip % 2].dma_start(out=dstl, in_=otl[:, :])
                dma_flip += 1
            assert g == NGROUP

    _split_multi_waits(nc)
    return nc


def _marshal_lg(xq: np.ndarray) -> np.ndarray:
    """Build the global lhsT input (N_CORES*128, NGROUP*128) int8 from the
    int8-quantized x.

    Layout per core: lg[k, g*128 + m], k=(s,u,v), groups ordered
    [A(b0),B(b0),...,A(b3),B(b3),C] per batch-quad; m=(c%4|j, f, h).
    x[b, c, 4f+u, 32h+4s+v]."""
    lgt = np.empty((N_CORES, 128, NQ, 9, 128), np.int8)
    lv = lgt.reshape(N_CORES, 8, P, P, NQ, 9, 128)  # k -> (s, u, v)
    # A/B groups: channels 0-7
    # axes: (core, bq, bl, ab, c4, f, u, h, s, v)
    xab = xq[:, 0:8].reshape(N_CORES, NQ, 4, 2, 4, NF, P, 2, 8, P)
    lv[:, :, :, :, :, 0:8, :] = (
        xab.transpose(0, 8, 6, 9, 1, 2, 3, 4, 5, 7)  # core,s,u,v,bq,bl,ab,c4,f,h
        .reshape(N_CORES, 8, P, P, NQ, 8, 128)
    )
    # C groups: channel 8, m=(j=batch lane, f, h)
    # axes: (core, bq, j, f, u, h, s, v)
    xc = xq[:, 8].reshape(N_CORES, NQ, 4, NF, P, 2, 8, P)
    lv[:, :, :, :, :, 8, :] = (
        xc.transpose(0, 6, 4, 7, 1, 2, 3, 5)  # core,s,u,v,bq,j,f,h
        .reshape(N_CORES, 8, P, P, NQ, 128)
    )
    return lgt.reshape(N_CORES * 128, NGROUP * 128)


def _marshal_lg_core(xq: np.ndarray, core: int) -> np.ndarray:
    """One core's slice of _marshal_lg: (128, NGROUP*128) int8."""
    sl = slice(BPC * core, BPC * (core + 1))
    lgt = np.empty((128, NQ, 9, 128), np.int8)
    lv = lgt.reshape(8, P, P, NQ, 9, 128)
    # (bq, bl, ab, c4, f, u, h, s, v) -> (s, u, v, bq, bl, ab, c4, f, h)
    xab = xq[sl, 0:8].reshape(NQ, 4, 2, 4, NF, P, 2, 8, P)
    lv[:, :, :, :, 0:8, :] = (
        xab.transpose(7, 5, 8, 0, 1, 2, 3, 4, 6).reshape(8, P, P, NQ, 8, 128)
    )
    # (bq, j, f, u, h, s, v) -> (s, u, v, bq, j, f, h)
    xc = xq[sl, 8].reshape(NQ, 4, NF, P, 2, 8, P)
    lv[:, :, :, :, 8, :] = (
        xc.transpose(5, 3, 6, 0, 1, 2, 4).reshape(8, P, P, NQ, 128)
    )
    return lgt.reshape(128, NGROUP * 128)


def _embed_table(b, channel_embed, spatial_embed, time_pos, freq_pos,
                 spatial_idx) -> np.ndarray:
    """emb[p=(c,f,t), e]: everything except the conv, f32 (2304, 192)."""
    chs = channel_embed + spatial_embed[spatial_idx] + b[None, :]  # (C, E)
    emb = (chs[:, None, None, :]
           + freq_pos[None, :, None, :]
           + time_pos[None, None, :, :])
    return np.ascontiguousarray(emb.reshape(N_PATCH, E), dtype=np.float32)


class _Exec:
    """One-time build: Bass module -> cached jitted PJRT executable."""

    def __init__(self):
        bass2jax.install_neuronx_cc_hook()
        nc = build_nc()
        self.nc = nc
        partition_name = (
            nc.partition_id_tensor.name if nc.partition_id_tensor else None
        )
        in_names: list[str] = []
        out_names: list[str] = []
        out_avals: list[jax.core.ShapedArray] = []
        for alloc in nc.m.functions[0].allocations:
            if not isinstance(alloc, mybir.MemoryLocationSet):
                continue
            name = alloc.memorylocations[0].name
            if alloc.kind == "ExternalInput":
                if name != partition_name:
                    in_names.append(name)
            elif alloc.kind == "ExternalOutput":
                out_names.append(name)
                out_avals.append(
                    jax.core.ShapedArray(
                        tuple(alloc.tensor_shape), mybir.dt.np(alloc.dtype)
                    )
                )
        n_params = len(in_names)
        n_outs = len(out_avals)
        self.in_names = list(in_names)
        self.out_names = list(out_names)
        in_names = in_names + out_names
        if partition_name is not None:
            in_names.append(partition_name)

        def _body(*args):
            operands = list(args)
            if partition_name is not None:
                operands.append(bass2jax.partition_id_tensor())
            outs = bass2jax._bass_exec_p.bind(
                *operands,
                out_avals=tuple(out_avals),
                in_names=tuple(in_names),
                out_names=tuple(out_names),
                lowering_input_output_aliases=(),
                sim_require_finite=True,
                sim_require_nnan=True,
                nc=nc,
            )
            return tuple(outs)

        devices = jax.devices()[:N_CORES]
        assert len(devices) == N_CORES, len(jax.devices())
        self.mesh = Mesh(np.asarray(devices), ("core",))
        spec = PartitionSpec("core")
        self.sharding = NamedSharding(self.mesh, spec)
        self.sharded = jax.jit(
            shard_map(
                _body,
                mesh=self.mesh,
                in_specs=(spec,) * (n_params + n_outs),
                out_specs=(spec,) * n_outs,
                check_rep=False,
            ),
            donate_argnums=tuple(range(n_params, n_params + n_outs)),
            keep_unused=True,
        )
        # donated output buffers, created on-device (no 85MB zero upload)
        self.zeros_fn = jax.jit(
            shard_map(
                lambda: (
                    jnp.zeros((BPC, N_PATCH, HB), jnp.uint8),
                    jnp.zeros((BPC, N_PATCH, LB), jnp.uint8),
                ),
                mesh=self.mesh,
                in_specs=(),
                out_specs=(spec, spec),
                check_rep=False,
            )
        )


_CACHE: dict = {}


def _get_exec() -> _Exec:
    if "exec" not in _CACHE:
        _CACHE["exec"] = _Exec()
        _CACHE["pool"] = ThreadPoolExecutor(N_CORES + 4)
    return _CACHE["exec"]


def kernel(**inputs: np.ndarray) -> np.ndarray:
    arrs = {k: np.asarray(v) for k, v in inputs.items()}
    x = arrs["x"].astype(np.float32, copy=False)
    assert x.shape == (B, C, FR, T), x.shape
    W = arrs["W"].astype(np.float32, copy=False)
    ex = _get_exec()
    pool = _CACHE["pool"]

    # input quantization xq = round(x / sx) and exact per-patch norms,
    # threaded over batch chunks (ufuncs release the GIL)
    xmax = float(max(x.max(), -x.min())) or 1.0
    sx = np.float32(xmax / 127.0)
    inv_sx = np.float32(1.0 / sx)
    xq = np.empty(x.shape, np.int8)
    pn2_parts = [0.0] * N_CORES

    def quant_chunk(i):
        sl = slice(BPC * i, BPC * (i + 1))
        xq[sl] = np.rint(x[sl] * inv_sx)

    def pn2_chunk(i):
        sl = slice(BPC * i, BPC * (i + 1))
        pn2 = np.square(x[sl]).reshape(BPC, C, NF, P, NT, P).sum(axis=(3, 5))
        pn2_parts[i] = float(pn2.max())

    qfuts = [pool.submit(quant_chunk, i) for i in range(N_CORES)]
    pfuts = [pool.submit(pn2_chunk, i) for i in range(N_CORES)]

    # 6-bit output scale via Cauchy-Schwarz: |conv| <= ||x_p|| * ||W_e||;
    # patch norms computed exactly, quantization slack of 0.5 per tap.
    wtb = W.transpose(1, 2, 0).reshape(P * P, E)  # [(u,v), e]
    wn_f = float(np.sqrt(np.square(wtb).sum(axis=0).max()))
    for f in pfuts:
        f.result()
    qn = float(np.sqrt(max(pn2_parts))) / float(sx) + 2.0  # max ||xq_p||_2
    s6 = np.float32(qn * float(sx) * wn_f / 30.9) if wn_f > 0 else np.float32(1.0)
    for _ in range(4):  # exact no-clip check on the bf16 weights actually sent
        wt16 = (wtb * (float(sx) / s6)).astype(ml_dtypes.bfloat16)
        wn = float(np.sqrt(np.square(wt16.astype(np.float32)).sum(axis=0).max()))
        if qn * wn <= 31.45:
            break
        s6 = np.float32(s6 * (qn * wn / 31.4))
    else:
        raise AssertionError("6-bit scale did not converge")
    wt_g = np.ascontiguousarray(
        np.broadcast_to(wt16, (N_CORES, 16, E)).reshape(N_CORES * 16, E)
    )
    for f in qfuts:
        f.result()

    # marshal, threaded per core into one buffer, then a single async put
    lg_full = np.empty((N_CORES * 128, NGROUP * 128), np.int8)

    def marshal_core(i):
        lg_full[128 * i : 128 * (i + 1)] = _marshal_lg_core(xq, i)

    for f in [pool.submit(marshal_core, i) for i in range(N_CORES)]:
        f.result()
    lg_dev = jax.device_put(lg_full, ex.sharding)

    emb = _embed_table(
        arrs["b"].astype(np.float32, copy=False),
        arrs["channel_embed"].astype(np.float32, copy=False),
        arrs["spatial_embed"].astype(np.float32, copy=False),
        arrs["time_pos"].astype(np.float32, copy=False),
        arrs["freq_pos"].astype(np.float32, copy=False),
        arrs["spatial_idx"],
    )
    s6f = np.float32(s6)
    emb_off = emb - np.float32(32.0) * s6f  # folds the +32 code offset

    zpair = _CACHE.pop("zeros_next", None) or ex.zeros_fn()
    hi_g, lo_g = ex.sharded(lg_dev, wt_g, *zpair)
    # donated buffers for the NEXT call, created while this call fetches
    _CACHE["zeros_next"] = ex.zeros_fn()

    # fetch shards in flight; decode with GIL-releasing ufunc arithmetic.
    # Keep one .data wrapper per shard (host-copy caching is per wrapper)
    # and issue async copies as (hi, lo) pairs per core so each core's
    # decode can start as soon as its pair lands.
    hi_arrs = {s.index[0].start or 0: s.data for s in hi_g.addressable_shards}
    lo_arrs = {s.index[0].start or 0: s.data for s in lo_g.addressable_shards}
    for i0 in sorted(hi_arrs):
        hi_arrs[i0].copy_to_host_async()
        lo_arrs[i0].copy_to_host_async()

    final = np.empty((B, N_PATCH, E), np.float32)
    CH = 8  # batches per decode task
    pool = _CACHE["pool"]

    def decode(ah, al, i0):
        hi = ah  # (CH, N_PATCH, HB) uint8, contiguous
        lo = al  # (CH, N_PATCH, LB)
        he = hi[..., 0::2]  # covers e%4 in {0,1}
        ho = hi[..., 1::2]  # covers e%4 in {2,3}
        Ub = np.empty((ah.shape[0], N_PATCH, E), np.uint8)
        t = np.right_shift(he, 4)
        np.left_shift(t, 2, out=t)
        t2 = np.right_shift(lo, 6)
        np.add(t, t2, out=t)
        Ub[..., 0::4] = t
        np.bitwise_and(he, 15, out=t)
        np.left_shift(t, 2, out=t)
        np.right_shift(lo, 4, out=t2)
        np.bitwise_and(t2, 3, out=t2)
        np.add(t, t2, out=t)
        Ub[..., 1::4] = t
        np.right_shift(ho, 4, out=t)
        np.left_shift(t, 2, out=t)
        np.right_shift(lo, 2, out=t2)
        np.bitwise_and(t2, 3, out=t2)
        np.add(t, t2, out=t)
        Ub[..., 2::4] = t
        np.bitwise_and(ho, 15, out=t)
        np.left_shift(t, 2, out=t)
        np.bitwise_and(lo, 3, out=t2)
        np.add(t, t2, out=t)
        Ub[..., 3::4] = t
        out_v = final[i0 : i0 + ah.shape[0]]
        np.multiply(Ub, s6f, out=out_v, casting="unsafe")
        np.add(out_v, emb_off[None], out=out_v)

    decode_futs = []

    def fetch(i0):
        ah = np.asarray(hi_arrs[i0])
        al = np.asarray(lo_arrs[i0])
        for j0 in range(0, BPC, CH):
            decode_futs.append(
                pool.submit(decode, ah[j0 : j0 + CH], al[j0 : j0 + CH], i0 + j0)
            )

    for f in [pool.submit(fetch, i0) for i0 in sorted(hi_arrs)]:
        f.result()
    for f in decode_futs:
        f.result()
    return final
